# revision 6
# baseline (speedup 1.0000x reference)
"""AutoSparse forward kernel for Trainium2 (8 NeuronCores, SPMD).

Computes out = sign(W) * relu(|W| - sigmoid(threshold)) for
W: [4096, 8192] f32, threshold: [4096, 1] f32 (row-broadcast).

Identity used on-device:  sign(w)*relu(|w|-s) == w - clamp(w, -s, s),
which is 2 DVE ops per tile (one 2x-mode tensor_scalar + one
tensor_tensor subtract) — the kernel is DMA/HBM-bound.

Sharding: rows split evenly across 8 cores (512 rows each); purely
elementwise per-row, so no collectives are needed.
"""

import numpy as np

import concourse.bass as bass
import concourse.tile as tile
from concourse import mybir
from concourse.bass_utils import run_bass_kernel_spmd

O, F = 4096, 8192
N_CORES = 8
ROWS = O // N_CORES          # 512 rows per core
P = 128                      # SBUF partitions
GROUPS = ROWS // P           # 4 row groups per core
COL_TILE = 4096              # 2 MiB f32 tiles per DMA
COL_TILES = F // COL_TILE

_FP32 = mybir.dt.float32


def _split_multi_waits(nc):
    """The walrus codegen in this container accepts at most ONE sync wait
    per instruction ("Too many sync wait commands"). Hoist all but the last
    wait of any multi-wait instruction into standalone same-engine
    InstEventSemaphore ops (the exact encoding raw-bass wait_ge uses)."""
    cnt = 0
    for fn in nc.m.functions:
        for b in fn.blocks:
            new = []
            for ins in b.instructions:
                si = ins.sync_info
                if si is not None and len(si.on_wait) > 1:
                    waits = list(si.on_wait)
                    for w in waits[:-1]:
                        cnt += 1
                        new.append(
                            mybir.InstEventSemaphore(
                                name=f"WSPLIT-{cnt}",
                                engine=ins.engine,
                                sync_info=mybir.SyncInfo(
                                    on_wait=[w], on_update=[]
                                ),
                            )
                        )
                    ins.sync_info = mybir.SyncInfo(
                        on_wait=[waits[-1]], on_update=list(si.on_update)
                    )
                new.append(ins)
            try:
                b.instructions = new
            except Exception:
                b.instructions[:] = new
    return nc


def _build_bass():
    nc = bass.Bass()
    w = nc.declare_dram_parameter("weight", [ROWS, F], _FP32, isOutput=False)
    th = nc.declare_dram_parameter("threshold", [ROWS, 1], _FP32, isOutput=False)
    out = nc.declare_dram_parameter("out", [ROWS, F], _FP32, isOutput=True)

    with tile.TileContext(nc) as tc:
        with (
            tc.tile_pool(name="const", bufs=1) as constp,
            tc.tile_pool(name="w", bufs=3) as wp,
            tc.tile_pool(name="c", bufs=3) as cp,
            tc.tile_pool(name="o", bufs=3) as op,
        ):
            # Per-row threshold prep: s = sigmoid(th), ns = -s, laid out as
            # [128, GROUPS] (column g holds rows g*128 .. g*128+127).
            th_t = constp.tile([P, GROUPS], _FP32)
            nc.sync.dma_start(
                out=th_t, in_=th.rearrange("(g p) one -> p (g one)", p=P)
            )
            s = constp.tile([P, GROUPS], _FP32)
            nc.scalar.activation(
                out=s, in_=th_t, func=mybir.ActivationFunctionType.Sigmoid
            )
            # ns = -s on ACT too, so both scalar sources live in one sem domain.
            ns = constp.tile([P, GROUPS], _FP32)
            nc.scalar.mul(ns, s, -1.0)
            # Warm-up TS: forces the DVE sequencer to observe ACT's s/ns once,
            # so the hot-loop TensorScalarPtr ops carry only their load-DMA
            # wait (the TS/ACT instruction structs fit a single sync wait).
            warm = constp.tile([P, 1], _FP32)
            nc.vector.tensor_scalar(
                out=warm,
                in0=s[:, 0:1],
                scalar1=ns[:, 0:1],
                scalar2=None,
                op0=mybir.AluOpType.add,
            )

            for g in range(GROUPS):
                rows = slice(g * P, (g + 1) * P)
                for t in range(COL_TILES):
                    cols = slice(t * COL_TILE, (t + 1) * COL_TILE)
                    wt = wp.tile([P, COL_TILE], _FP32)
                    nc.sync.dma_start(out=wt, in_=w[rows, cols])
                    # c = clamp(w, -s, s)  (2x-mode tensor_scalar)
                    ct = cp.tile([P, COL_TILE], _FP32)
                    nc.vector.tensor_scalar(
                        out=ct,
                        in0=wt,
                        scalar1=ns[:, g : g + 1],
                        scalar2=s[:, g : g + 1],
                        op0=mybir.AluOpType.max,
                        op1=mybir.AluOpType.min,
                    )
                    # out = w - c
                    ot = op.tile([P, COL_TILE], _FP32)
                    nc.vector.tensor_sub(ot, wt, ct)
                    # Stores on the ACT HWDGE ring, loads on the SP ring.
                    nc.scalar.dma_start(out=out[rows, cols], in_=ot)
    return _split_multi_waits(nc)


_nc_cache = None


def _get_nc():
    global _nc_cache
    if _nc_cache is None:
        _nc_cache = _build_bass()
    return _nc_cache


def kernel(weight, threshold, trace=False):
    weight = np.ascontiguousarray(np.asarray(weight, dtype=np.float32))
    threshold = np.ascontiguousarray(np.asarray(threshold, dtype=np.float32))
    assert weight.shape == (O, F) and threshold.shape == (O, 1)

    nc = _get_nc()
    in_maps = [
        {
            "weight": weight[i * ROWS : (i + 1) * ROWS],
            "threshold": threshold[i * ROWS : (i + 1) * ROWS],
        }
        for i in range(N_CORES)
    ]
    kwargs = {}
    if trace:
        import os

        tdir = os.path.abspath("trace_out")
        os.makedirs(tdir, exist_ok=True)
        for f in os.listdir(tdir):
            os.remove(os.path.join(tdir, f))
        os.environ["KEEP_NEFF_DIR"] = tdir
        kwargs["tmpdir"] = tdir
    res = run_bass_kernel_spmd(
        nc, in_maps, list(range(N_CORES)), trace=trace, **kwargs
    )
    full = np.concatenate([res.results[i]["out"] for i in range(N_CORES)], axis=0)
    if trace:
        return full, res
    return full
